# revision 18
# baseline (speedup 1.0000x reference)
"""Trainium2 Bass kernel for nn_ArchDecoder: two stacked LSTM cells (H=2048,
H=4096) unrolled DEPTH=12 sequential steps, batch=1, tensor-parallel across 8
NeuronCores.

v3 design:
- Two AllGathers per step: a small A-gather carrying the arch-LSTM state
  [h_a | arch-logit-partials] (the arch chain runs ~2 steps ahead), and a
  B-gather carrying [c_hp | h_hp | hp-logit-partials].
- Deferred softmax normalization: unnormalized exp(logits) feed the next
  step's matvecs (linear); separate PSUM groups are scaled by 1/sum at
  combine time, so the cross-partition sum/reciprocal is off the critical
  path.
- No Sigmoid: sigma(x) = 0.5*tanh(0.5x)+0.5, so exp/tanh/relu share one ACT
  table -> zero ACT_TABLE_LOAD per step.
- PE stream per step: [g2a, g2b, whpc, wsum, ga(t+2) m0..3, lhp, ga m4..7 +
  ga2, woa] — the hp-input matvecs run before whpc so the gate combine only
  gates on whpc's end; the arch block is split around lhp so the payload DMA
  fires as soon as the hp tail is done while the PE keeps crunching arch
  work into the collective's latency shadow.
- Gate-combine partial sums (inp-scale + bias) are emitted before whpc so
  only one DVE add sits after the whpc stream.
- PSUM tiles are never shared across producer constructs with different
  completion times (avoids false waits on the payload path).
- whpc is loaded as 4 m-range pieces and wa as 2, spread over all three
  DMA-capable queues, so early steps overlap the 26MB weight load; a dummy
  collective fires first to absorb the ~50us cold-CC barrier.
- Outputs accumulate in SBUF; one contiguous [128, 48] DMA at the end.

Gate packing order is [i, f, o, g] so the three sigmoids are contiguous.
"""
import sys

for _p in ("/opt/trn_rl_repo", "/root/.axon_site", "/root/.axon_site/_ro/pypackages"):
    if _p not in sys.path:
        sys.path.insert(0, _p)

import numpy as np
import ml_dtypes

import concourse.bass as bass
import concourse.bacc as bacc
import concourse.mybir as mybir
import concourse.tile as tile
from concourse import bass_isa
from concourse.bass_utils import run_bass_kernel_spmd

NC = 8
V = 256
HA = 2048
HHP = 4096
DEPTH = 12
BF = mybir.dt.bfloat16
F32 = mybir.dt.float32
FP8 = mybir.dt.float8e4
AF = mybir.ActivationFunctionType
ALU = mybir.AluOpType

SA = HA // NC          # 256 h_a positions per core
SS = HA // NC          # 256 h_sum positions per core
SHP = SA + SS          # 512 hp-state positions per core
MA = 4 * SA // 128     # 8  M-tiles for arch gates
MHP = 4 * SHP // 128   # 16 M-tiles for hp gates
KA = (V + HA) // 128   # 18 K-chunks for arch gates ([a ; h_a])
KHP_C = HHP // 128     # 32 c_hp K-chunks
KHP_I = (2 * V) // 128 # 4 inp K-chunks
KSUM = HHP // 128      # 32
CA = 4                 # A payload cols: [h_a(2) | lpA(2)]
CB = 10                # B payload cols: [c(4) | h(4) | lpB(2)]
NPC = 4                # whpc split into 4 m-range pieces
MPP = MHP // NPC       # 4 m-tiles per whpc piece

_acol = lambda kc: (kc // 2) * CA + (kc % 2)
_ccol = lambda kc: (kc // 4) * CB + (kc % 4)
_hcol = lambda kc: (kc // 4) * CB + 4 + (kc % 4)


def _build_nc():
    nc = bacc.Bacc(None, target_bir_lowering=False, num_devices=NC)

    wa_e = [nc.declare_dram_parameter(f"wa{p}", [128, 4 * KA * 128], BF,
                                      isOutput=False) for p in range(2)]
    whpc_e = [nc.declare_dram_parameter(f"whpc{p}", [128, MPP * KHP_C * 128], FP8,
                                        isOutput=False) for p in range(NPC)]
    wsum_e = nc.declare_dram_parameter("wsum", [128, 2 * KSUM * 128], BF, isOutput=False)
    whpi_e = nc.declare_dram_parameter("whpi", [128, MHP * KHP_I * 128], BF, isOutput=False)
    woa_e = nc.declare_dram_parameter("woa", [128, 2 * 2 * 128], BF, isOutput=False)
    wohp_e = nc.declare_dram_parameter("wohp", [128, 2 * 4 * 128], BF, isOutput=False)
    ba_e = nc.declare_dram_parameter("ba", [128, MA], F32, isOutput=False)
    bsum_e = nc.declare_dram_parameter("bsum", [128, 2], F32, isOutput=False)
    bhp_e = nc.declare_dram_parameter("bhp", [128, MHP], F32, isOutput=False)
    boa8_e = nc.declare_dram_parameter("boa8", [128, 2], F32, isOutput=False)
    bohp8_e = nc.declare_dram_parameter("bohp8", [128, 2], F32, isOutput=False)
    ident_e = nc.declare_dram_parameter("ident", [128, 128], BF, isOutput=False)
    initA_e = nc.declare_dram_parameter("initA", [128, NC * CA], BF, isOutput=False)
    initB_e = nc.declare_dram_parameter("initB", [128, NC * CB], BF, isOutput=False)
    out_e = nc.declare_dram_parameter("out", [128, 4 * DEPTH], F32, isOutput=True)

    with tile.TileContext(nc, num_cores=NC) as tc:
        with (
            tc.tile_pool(name="wpool", bufs=1) as wpool,
            tc.tile_pool(name="cpool", bufs=1) as cpool,
            tc.tile_pool(name="apool", bufs=4) as apool,
            tc.tile_pool(name="bpool", bufs=4) as bpool,
            tc.tile_pool(name="spool", bufs=4) as spool,
            tc.tile_pool(name="xpool", bufs=3) as xpool,
            tc.tile_pool(name="psA", bufs=1, space="PSUM") as psA,
            tc.tile_pool(name="psGH", bufs=2, space="PSUM") as psGH,
            tc.tile_pool(name="psI", bufs=1, space="PSUM") as psI,
            tc.tile_pool(name="psT", bufs=1, space="PSUM") as psT,
            tc.tile_pool(name="dramA", bufs=3, space="DRAM") as dramA,
            tc.tile_pool(name="dramB", bufs=3, space="DRAM") as dramB,
            tc.tile_pool(name="dramW", bufs=1, space="DRAM") as dramW,
        ):
            # ---- warm collectives first: their tiny input DMA is issued
            # before any weight DMA, so the triggers fire within ~1us and the
            # ~50us cold-CC barrier overlaps the weight load ----
            warm = cpool.tile([1, 16], BF, tag="warm")
            nc.vector.memset(warm[:], 0.0)
            warm_d = dramW.tile([1, 16], BF, tag="warm_d")
            nc.sync.dma_start(warm_d[:], warm[:])
            gw_d = dramW.tile([NC, 1, 16], BF, tag="gw_d")
            nc.gpsimd.collective_compute(
                "AllGather", ALU.bypass,
                replica_groups=[list(range(NC))],
                ins=[warm_d.opt()], outs=[gw_d.opt()],
            )

            # ---- weights: pieces spread over the three DMA queues ----
            initA_sb = cpool.tile([128, NC * CA], BF, tag="initA")
            initB_sb = cpool.tile([128, NC * CB], BF, tag="initB")
            nc.sync.dma_start(initA_sb[:], initA_e[:])
            nc.sync.dma_start(initB_sb[:], initB_e[:])
            wa = [wpool.tile([128, 4 * KA * 128], BF, tag=f"wa{p}", name=f"wa{p}") for p in range(2)]
            whpc = [wpool.tile([128, MPP * KHP_C * 128], FP8, tag=f"whpc{p}",
                                name=f"whpc{p}") for p in range(NPC)]
            wsum = wpool.tile([128, 2 * KSUM * 128], BF, tag="wsum")
            whpi = wpool.tile([128, MHP * KHP_I * 128], BF, tag="whpi")
            woa = wpool.tile([128, 2 * 2 * 128], BF, tag="woa")
            wohp = wpool.tile([128, 2 * 4 * 128], BF, tag="wohp")
            ba = cpool.tile([128, MA], F32, tag="ba")
            bsum = cpool.tile([128, 2], F32, tag="bsum")
            bhp = cpool.tile([128, MHP], F32, tag="bhp")
            boa8 = cpool.tile([128, 2], F32, tag="boa8")
            bohp8 = cpool.tile([128, 2], F32, tag="bohp8")
            idt = cpool.tile([128, 128], BF, tag="idt")
            nc.sync.dma_start(ba[:], ba_e[:])
            nc.sync.dma_start(bsum[:], bsum_e[:])
            nc.sync.dma_start(bhp[:], bhp_e[:])
            nc.sync.dma_start(boa8[:], boa8_e[:])
            nc.sync.dma_start(bohp8[:], bohp8_e[:])
            nc.sync.dma_start(idt[:], ident_e[:])
            nc.sync.dma_start(woa[:], woa_e[:])
            nc.sync.dma_start(wohp[:], wohp_e[:])
            nc.sync.dma_start(wa[0][:], wa_e[0][:])
            nc.sync.dma_start(wa[1][:], wa_e[1][:])
            nc.sync.dma_start(wsum[:], wsum_e[:])
            nc.sync.dma_start(whpi[:], whpi_e[:])
            nc.sync.dma_start(whpc[0][:], whpc_e[0][:])
            nc.sync.dma_start(whpc[1][:], whpc_e[1][:])
            nc.sync.dma_start(whpc[2][:], whpc_e[2][:])
            nc.sync.dma_start(whpc[3][:], whpc_e[3][:])

            a0_bf = cpool.tile([128, 2], BF, tag="a0_bf")
            ahp0_bf = cpool.tile([128, 2], BF, tag="ahp0_bf")
            one_s = cpool.tile([128, 1], F32, tag="one_s")
            nc.vector.memset(a0_bf[:], 1.0 / V)
            nc.vector.memset(ahp0_bf[:], 1.0 / V)
            nc.vector.memset(one_s[:], 1.0)
            c_a = cpool.tile([128, 2], F32, tag="c_a")
            nc.vector.memset(c_a[:], 0.0)
            outA = cpool.tile([128, 2 * DEPTH], F32, tag="outA")
            outHP = cpool.tile([128, 2 * DEPTH], F32, tag="outHP")

            def A_mm1(ea_bf, allA_prev, m_lo, m_hi, ga=None):
                """ga1 MMs for arch m-tiles [m_lo, m_hi); allocates ga on demand.
                ga tile cols: ga1 [0:8], ga2 [8:16], la [16:18]."""
                if ga is None:
                    ga = psA.tile([128, 2 * MA + 2], F32, tag="ga")
                for m in range(m_lo, m_hi):
                    w = wa[m // 4]
                    mm = m % 4
                    for kc in range(16):
                        nc.tensor.matmul(
                            ga[:, m:m + 1],
                            w[:, (mm * KA + 2 + kc) * 128:(mm * KA + 2 + kc + 1) * 128],
                            allA_prev[:, _acol(kc):_acol(kc) + 1],
                            start=(kc == 0), stop=(kc == 15),
                        )
                return ga

            def A_rest(t, ga, ea_bf, Ra, allA_prev):
                """ga2 MMs, arch acts, payA, woa, gather-A(t)."""
                for m in range(MA):
                    w = wa[m // 4]
                    mm = m % 4
                    for j in range(2):
                        nc.tensor.matmul(
                            ga[:, MA + m:MA + m + 1],
                            w[:, (mm * KA + j) * 128:(mm * KA + j + 1) * 128],
                            ea_bf[:, j:j + 1],
                            start=(j == 0), stop=(j == 1),
                        )
                g2s = spool.tile([128, MA], F32, tag="g2s")
                nc.vector.tensor_scalar(g2s[:], ga[:, MA:2 * MA], Ra[:, 0:1], None, ALU.mult)
                nc.vector.tensor_add(g2s[:], g2s[:], ba[:])
                gsb = spool.tile([128, MA], F32, tag="gsb")
                nc.vector.tensor_add(gsb[:], g2s[:], ga[:, 0:MA])
                # gate cols: i=[0:2] f=[2:4] o=[4:6] g=[6:8]
                th = spool.tile([128, MA], F32, tag="th")
                nc.scalar.activation(th[:, 0:6], gsb[:, 0:6], AF.Tanh, scale=0.5)
                nc.scalar.activation(th[:, 6:8], gsb[:, 6:8], AF.Tanh)
                sg = spool.tile([128, 6], F32, tag="sg")
                nc.vector.tensor_scalar(sg[:], th[:, 0:6], 0.5, 0.5, ALU.mult, ALU.add)
                t1a = spool.tile([128, 2], F32, tag="t1a")
                t2a = spool.tile([128, 2], F32, tag="t2a")
                nc.vector.tensor_mul(t1a[:], sg[:, 0:2], th[:, 6:8])
                nc.vector.tensor_mul(t2a[:], sg[:, 2:4], c_a[:])
                nc.vector.tensor_add(c_a[:], t1a[:], t2a[:])
                tc_a = spool.tile([128, 2], F32, tag="tc_a")
                nc.scalar.activation(tc_a[:], c_a[:], AF.Tanh)
                ha_f = apool.tile([128, 2], F32, tag="ha_f")
                nc.vector.tensor_mul(ha_f[:], sg[:, 4:6], tc_a[:])
                payA = apool.tile([128, CA], BF, tag="payA")
                nc.vector.tensor_copy(payA[:, 0:2], ha_f[:])
                for m in range(2):
                    for kc in range(2):
                        nc.tensor.matmul(
                            ga[:, 2 * MA + m:2 * MA + m + 1],
                            woa[:, (m * 2 + kc) * 128:(m * 2 + kc + 1) * 128],
                            payA[:, kc:kc + 1], start=(kc == 0), stop=(kc == 1),
                        )
                nc.vector.tensor_add(payA[:, 2:4], ga[:, 2 * MA:2 * MA + 2], boa8[:])
                tpa = psA.tile([128, 160], BF, tag="tpa")
                tpSA = tpa[0:CA, 32:160]
                nc.tensor.transpose(tpSA, payA[:], idt[:])
                pat = apool.tile([CA, 128], BF, tag="pat")
                nc.vector.tensor_copy(pat[:], tpSA)
                payA_d = dramA.tile([CA, 128], BF, tag="payA_d")
                gA_d = dramA.tile([NC, CA, 128], BF, tag="gA_d")
                nc.sync.dma_start(payA_d[:], pat[:])
                nc.gpsimd.collective_compute(
                    "AllGather", ALU.bypass,
                    replica_groups=[list(range(NC))],
                    ins=[payA_d.opt()], outs=[gA_d.opt()],
                )
                return ha_f, gA_d

            def readback_A(gA_d):
                rbA = xpool.tile([NC * CA, 128], BF, tag="rbA")
                nc.sync.dma_start(rbA[:], gA_d[:].rearrange("r c p -> (r c) p"))
                tpa = psA.tile([128, 160], BF, tag="tpa")
                tpRA = tpa[:, 0:NC * CA]
                nc.tensor.transpose(tpRA, rbA[:], idt[0:NC * CA, 0:NC * CA])
                allA = xpool.tile([128, NC * CA], BF, tag="allA")
                nc.vector.tensor_copy(allA[:], tpRA)
                return allA

            def H2(t, allA_t):
                """exp of arch logits t (unnormalized), 1/sum, outA[:, t]."""
                red = spool.tile([128, 2], F32, tag="redA")
                nc.vector.tensor_reduce(
                    red[:],
                    allA_t[:].rearrange("p (r c) -> p c r", r=NC)[:, 2:4, :],
                    mybir.AxisListType.X, ALU.add,
                )
                e_f = spool.tile([128, 2], F32, tag="eA_f")
                sp = spool.tile([128, 1], F32, tag="sA_p")
                nc.scalar.activation(e_f[:], red[:], AF.Exp, accum_out=sp[:])
                ea_bf = apool.tile([128, 2], BF, tag="ea_bf")
                nc.vector.tensor_copy(ea_bf[:], e_f[:])
                ssum = spool.tile([128, 1], F32, tag="sA_t")
                nc.gpsimd.partition_all_reduce(ssum[:], sp[:], 128, bass_isa.ReduceOp.add)
                Ra = apool.tile([128, 1], F32, tag="Ra")
                nc.vector.reciprocal(Ra[:], ssum[:])
                nc.vector.tensor_scalar(
                    outA[:, 2 * t:2 * t + 2], e_f[:], Ra[:, 0:1], None, ALU.mult)
                return ea_bf, Ra

            def H1(t, allB_t):
                """exp of hp logits t (unnormalized), 1/sum, outHP[:, t]."""
                red = spool.tile([128, 2], F32, tag="redB")
                nc.vector.tensor_reduce(
                    red[:],
                    allB_t[:].rearrange("p (r c) -> p c r", r=NC)[:, 8:10, :],
                    mybir.AxisListType.X, ALU.add,
                )
                e_f = spool.tile([128, 2], F32, tag="eB_f")
                sp = spool.tile([128, 1], F32, tag="sB_p")
                nc.scalar.activation(e_f[:], red[:], AF.Exp, accum_out=sp[:])
                ehp_bf = bpool.tile([128, 2], BF, tag="ehp_bf")
                nc.vector.tensor_copy(ehp_bf[:], e_f[:])
                ssum = spool.tile([128, 1], F32, tag="sB_t")
                nc.gpsimd.partition_all_reduce(ssum[:], sp[:], 128, bass_isa.ReduceOp.add)
                Rhp = bpool.tile([128, 1], F32, tag="Rhp")
                nc.vector.reciprocal(Rhp[:], ssum[:])
                nc.vector.tensor_scalar(
                    outHP[:, 2 * t:2 * t + 2], e_f[:], Rhp[:, 0:1], None, ALU.mult)
                return ehp_bf, Rhp

            def B_mm_head(t, allB_prev, ea_bf, ehp_bf):
                """PE: g2a, g2b, wsum, then whpc m-tiles [0, 8)."""
                gi = psI.tile([128, 2 * MHP], F32, tag="gi")
                for m in range(MHP):
                    for j in range(2):
                        nc.tensor.matmul(
                            gi[:, m:m + 1],
                            whpi[:, (m * KHP_I + j) * 128:(m * KHP_I + j + 1) * 128],
                            ea_bf[:, j:j + 1], start=(j == 0), stop=(j == 1),
                        )
                for m in range(MHP):
                    for j in range(2):
                        nc.tensor.matmul(
                            gi[:, MHP + m:MHP + m + 1],
                            whpi[:, (m * KHP_I + 2 + j) * 128:(m * KHP_I + 2 + j + 1) * 128],
                            ehp_bf[:, j:j + 1], start=(j == 0), stop=(j == 1),
                        )
                ghpA = psGH.tile([128, 8], F32, tag="ghpA")
                ghpB = psGH.tile([128, 12], F32, tag="ghpB")
                for m in range(2):
                    for kc in range(KSUM):
                        nc.tensor.matmul(
                            ghpB[:, 8 + m:9 + m],
                            wsum[:, (m * KSUM + kc) * 128:(m * KSUM + kc + 1) * 128],
                            allB_prev[:, _hcol(kc):_hcol(kc) + 1],
                            start=(kc == 0), stop=(kc == KSUM - 1),
                        )
                if t > 0:
                    B_whpc_half(t, allB_prev, ghpA, 0)
                return ghpA, ghpB, gi

            def B_whpc_half(t, allB_prev, ghp, h):
                for m in range(8 * h, 8 * h + 8):
                    w = whpc[m // MPP]
                    mm = m % MPP
                    for kc in range(KHP_C):
                        nc.tensor.matmul(
                            ghp[:, m - 8 * h:m - 8 * h + 1],
                            w[:, (mm * KHP_C + kc) * 128:(mm * KHP_C + kc + 1) * 128],
                            allB_prev[:, _ccol(kc):_ccol(kc) + 1],
                            start=(kc == 0), stop=(kc == KHP_C - 1),
                        )

            def B_relu(ghpB):
                hs_sb = bpool.tile([128, 2], F32, tag="hs_sb")
                nc.scalar.activation(hs_sb[:, 0:1], ghpB[:, 8:9], AF.Relu,
                                     bias=bsum[:, 0:1])
                nc.scalar.activation(hs_sb[:, 1:2], ghpB[:, 9:10], AF.Relu,
                                     bias=bsum[:, 1:2])
                return hs_sb

            def B_acts_half(t, ghp, gi, ha_f, hs_sb, Ra, Rhp, payB, con, h):
                """hp gate tail for state cols [2h:2h+2] (gates packed
                half-major: m-tiles [8h:8h+8] = [i(2) f(2) o(2) g(2)])."""
                o = 8 * h
                u1 = spool.tile([128, 8], F32, tag=f"u1_{h}", name=f"u1_{h}")
                u2 = spool.tile([128, 8], F32, tag=f"u2_{h}", name=f"u2_{h}")
                nc.vector.tensor_scalar(u1[:], gi[:, o:o + 8], Ra[:, 0:1],
                                        None, ALU.mult)
                nc.vector.tensor_scalar(u2[:], gi[:, MHP + o:MHP + o + 8],
                                        Rhp[:, 0:1], None, ALU.mult)
                gsb = spool.tile([128, 8], F32, tag=f"gsb_{h}", name=f"gsb_{h}")
                nc.vector.tensor_add(gsb[:], u1[:], u2[:])
                nc.vector.tensor_add(gsb[:], gsb[:], bhp[:, o:o + 8])
                if t > 0:
                    nc.vector.tensor_add(gsb[:], gsb[:], ghp[:, 0:8])
                # half-local gate cols: i=[0:2] f=[2:4] o=[4:6] g=[6:8]
                th = spool.tile([128, 8], F32, tag=f"th_{h}", name=f"th_{h}")
                nc.scalar.activation(th[:, 0:6], gsb[:, 0:6], AF.Tanh, scale=0.5)
                nc.scalar.activation(th[:, 6:8], gsb[:, 6:8], AF.Tanh)
                sg = spool.tile([128, 6], F32, tag=f"sg_{h}", name=f"sg_{h}")
                nc.vector.tensor_scalar(sg[:], th[:, 0:6], 0.5, 0.5,
                                        ALU.mult, ALU.add)
                t1 = spool.tile([128, 2], F32, tag=f"t1_{h}", name=f"t1_{h}")
                t2 = spool.tile([128, 2], F32, tag=f"t2_{h}", name=f"t2_{h}")
                nc.vector.tensor_mul(t1[:], sg[:, 0:2], th[:, 6:8])
                nc.vector.tensor_mul(t2[:], sg[:, 2:4],
                                     ha_f[:] if h == 0 else hs_sb[:])
                nc.vector.tensor_add(con[:, 2 * h:2 * h + 2], t1[:], t2[:])
                nc.vector.tensor_copy(payB[:, 2 * h:2 * h + 2],
                                      con[:, 2 * h:2 * h + 2])
                tch = spool.tile([128, 2], F32, tag=f"tch_{h}", name=f"tch_{h}")
                nc.scalar.activation(tch[:], con[:, 2 * h:2 * h + 2], AF.Tanh)
                nc.vector.tensor_mul(payB[:, 4 + 2 * h:6 + 2 * h],
                                     sg[:, 4:6], tch[:])

            def B_fin(t, payB, ghpB):
                """lhp logits, transpose payB to [CB,128], send, gather-B(t)."""
                for m in range(2):
                    for kc in range(4):
                        nc.tensor.matmul(
                            ghpB[:, 10 + m:11 + m],
                            wohp[:, (m * 4 + kc) * 128:(m * 4 + kc + 1) * 128],
                            payB[:, 4 + kc:5 + kc], start=(kc == 0), stop=(kc == 3),
                        )
                nc.vector.tensor_add(payB[:, 8:10], ghpB[:, 10:12], bohp8[:])
                tp = psT.tile([128, 208], BF, tag="tp")
                tpS = tp[0:CB, 80:208]
                nc.tensor.transpose(tpS, payB[:], idt[:])
                pbt = bpool.tile([CB, 128], BF, tag="pbt")
                nc.vector.tensor_copy(pbt[:], tpS)
                payB_d = dramB.tile([CB, 128], BF, tag="payB_d")
                gB_d = dramB.tile([NC, CB, 128], BF, tag="gB_d")
                nc.sync.dma_start(payB_d[:], pbt[:])
                nc.gpsimd.collective_compute(
                    "AllGather", ALU.bypass,
                    replica_groups=[list(range(NC))],
                    ins=[payB_d.opt()], outs=[gB_d.opt()],
                )
                return gB_d

            def readback_B(gB_d):
                rb = xpool.tile([NC * CB, 128], BF, tag="rb")
                nc.sync.dma_start(rb[:], gB_d[:].rearrange("r c p -> (r c) p"))
                tp = psT.tile([128, 208], BF, tag="tp")
                tpR = tp[:, 0:NC * CB]
                nc.tensor.transpose(tpR, rb[:], idt[0:NC * CB, 0:NC * CB])
                allB = xpool.tile([128, NC * CB], BF, tag="allB")
                nc.vector.tensor_copy(allB[:], tpR)
                return allB

            # ---- prologue: arch chain two steps ahead ----
            haf = {}
            eaD = {}
            RaD = {}
            ga0 = A_mm1(a0_bf, initA_sb, 0, MA)
            haf[0], gA_d0 = A_rest(0, ga0, a0_bf, one_s, initA_sb)
            allA_cur = readback_A(gA_d0)
            eaD[0], RaD[0] = H2(0, allA_cur)
            ga1_ = A_mm1(eaD[0], allA_cur, 0, MA)
            haf[1], gA_pending = A_rest(1, ga1_, eaD[0], RaD[0], allA_cur)

            allB_prev = initB_sb
            ehp, Rhp = ahp0_bf, one_s
            for t in range(DEPTH):
                if gA_pending is not None:
                    allA_cur = readback_A(gA_pending)
                    gA_pending = None
                ghpA, ghpB, gi = B_mm_head(t, allB_prev, eaD[t], ehp)
                hs_sb = B_relu(ghpB)
                payB = bpool.tile([128, CB], BF, tag="payB")
                con = bpool.tile([128, 4], F32, tag="con")
                B_acts_half(t, ghpA, gi, haf[t], hs_sb, RaD[t], Rhp, payB, con, 0)
                if t > 0:
                    B_whpc_half(t, allB_prev, ghpB, 1)
                if t + 1 < DEPTH:
                    eaD[t + 1], RaD[t + 1] = H2(t + 1, allA_cur)
                B_acts_half(t, ghpB, gi, haf[t], hs_sb, RaD[t], Rhp, payB, con, 1)
                ga = None
                if t + 2 < DEPTH:
                    ga = A_mm1(eaD[t + 1], allA_cur, 0, 4)
                gB_d = B_fin(t, payB, ghpB)
                if t + 2 < DEPTH:
                    ga = A_mm1(eaD[t + 1], allA_cur, 4, MA, ga=ga)
                    haf[t + 2], gA_pending = A_rest(
                        t + 2, ga, eaD[t + 1], RaD[t + 1], allA_cur)
                allB_new = readback_B(gB_d)
                ehp, Rhp = H1(t, allB_new)
                allB_prev = allB_new

            nc.sync.dma_start(out_e[:, 0:2 * DEPTH], outA[:])
            nc.sync.dma_start(out_e[:, 2 * DEPTH:4 * DEPTH], outHP[:])
    nc.finalize()
    return nc


_NC_CACHE = None


def _get_nc():
    global _NC_CACHE
    if _NC_CACHE is None:
        _NC_CACHE = _build_nc()
    return _NC_CACHE


def _lhsT_pack(w_cat, n_m, n_k):
    """w_cat [n_m*128 rows, n_k*128 cols] -> SBUF image [128, n_m*n_k*128] where
    cols [(m*n_k+kc)*128 + j] on partition p = w_cat[m*128 + j, kc*128 + p]."""
    a = w_cat.reshape(n_m, 128, n_k, 128)           # [m, j, kc, p]
    return np.ascontiguousarray(a.transpose(3, 0, 2, 1).reshape(128, n_m * n_k * 128))


GO = (0, 1, 3, 2)  # pack gate blocks in order i, f, o, g


def _prep_in_maps(x_thought_vec_arch, x_thought_vec_arch_hp,
                  W_ih_a, W_hh_a, b_ih_a, b_hh_a, W_out_a, b_out_a,
                  W_sum, b_sum, W_ih_hp, W_hh_hp, b_ih_hp, b_hh_hp,
                  W_out_hp, b_out_hp):
    f32 = np.float32
    bf16 = ml_dtypes.bfloat16
    php = np.concatenate([
        np.concatenate([np.arange(SA * k, SA * (k + 1)),
                        HA + np.arange(SS * k, SS * (k + 1))])
        for k in range(NC)
    ])
    ba_full = (np.asarray(b_ih_a) + np.asarray(b_hh_a)).astype(f32)
    bhp_full = (np.asarray(b_ih_hp) + np.asarray(b_hh_hp)).astype(f32)
    ha0 = np.asarray(x_thought_vec_arch, f32).reshape(HA)
    hhp0 = np.asarray(x_thought_vec_arch_hp, f32).reshape(HHP)
    W_ih_a = np.asarray(W_ih_a, f32); W_hh_a = np.asarray(W_hh_a, f32)
    W_out_a = np.asarray(W_out_a, f32); W_sum = np.asarray(W_sum, f32)
    W_ih_hp = np.asarray(W_ih_hp, f32); W_hh_hp = np.asarray(W_hh_hp, f32)
    W_out_hp = np.asarray(W_out_hp, f32)
    b_out_a = np.asarray(b_out_a, f32); b_out_hp = np.asarray(b_out_hp, f32)
    b_sum = np.asarray(b_sum, f32)

    initA = np.zeros((128, NC * CA), f32)
    initB = np.zeros((128, NC * CB), f32)
    hhp0_p = hhp0[php]
    for r in range(NC):
        for q in range(2):
            initA[:, r * CA + q] = ha0[r * SA + q * 128: r * SA + (q + 1) * 128]
        for q in range(4):
            initB[:, r * CB + 4 + q] = hhp0_p[r * SHP + q * 128: r * SHP + (q + 1) * 128]

    in_maps = []
    for k in range(NC):
        ja = np.arange(SA * k, SA * (k + 1))
        rows_a = np.concatenate([g * HA + ja for g in GO])
        wa_cat = np.concatenate([W_ih_a[rows_a], W_hh_a[rows_a]], axis=1)
        jhp = php[SHP * k: SHP * (k + 1)]
        # half-major gate packing: [i f o g] for state cols 0:2, then for 2:4
        rows_hp = np.concatenate([g * HHP + jhp[256 * h: 256 * (h + 1)]
                                  for h in range(2) for g in GO])
        whpc_cat = W_hh_hp[rows_hp][:, php]
        whpi_cat = W_ih_hp[rows_hp]
        js = np.arange(SS * k, SS * (k + 1))
        wsum_p = W_sum[js][:, php]
        woa_p = W_out_a[:, ja]
        wohp_p = W_out_hp[:, jhp]
        im = {
            "wsum": _lhsT_pack(wsum_p, 2, KSUM).astype(bf16),
            "whpi": _lhsT_pack(whpi_cat, MHP, KHP_I).astype(bf16),
            "woa": _lhsT_pack(woa_p, 2, 2).astype(bf16),
            "wohp": _lhsT_pack(wohp_p, 2, 4).astype(bf16),
            "ba": np.ascontiguousarray(ba_full[rows_a].reshape(MA, 128).T),
            "bsum": np.ascontiguousarray(b_sum[js].reshape(2, 128).T),
            "bhp": np.ascontiguousarray(bhp_full[rows_hp].reshape(MHP, 128).T),
            "boa8": np.ascontiguousarray((b_out_a / NC).reshape(2, 128).T),
            "bohp8": np.ascontiguousarray((b_out_hp / NC).reshape(2, 128).T),
            "ident": np.eye(128, dtype=bf16),
            "initA": initA.astype(bf16),
            "initB": initB.astype(bf16),
        }
        for p in range(2):
            im[f"wa{p}"] = _lhsT_pack(
                wa_cat[p * 512:(p + 1) * 512], 4, KA).astype(bf16)
        fp8 = ml_dtypes.float8_e4m3
        for p in range(NPC):
            im[f"whpc{p}"] = np.clip(_lhsT_pack(
                whpc_cat[p * MPP * 128:(p + 1) * MPP * 128], MPP, KHP_C),
                -240, 240).astype(fp8)
        in_maps.append(im)
    return in_maps


def _unpack_out(out):
    """out [128, 4*DEPTH] f32 -> (arch [1, DEPTH, V], arch_hp [1, DEPTH, V])."""
    out = np.asarray(out, np.float32)
    arch = out[:, :2 * DEPTH].reshape(128, DEPTH, 2).transpose(1, 2, 0).reshape(DEPTH, V)
    ahp = out[:, 2 * DEPTH:].reshape(128, DEPTH, 2).transpose(1, 2, 0).reshape(DEPTH, V)
    return arch[None], ahp[None]


def _run(in_maps, trace=False):
    nc = _get_nc()
    return run_bass_kernel_spmd(nc, in_maps, core_ids=list(range(NC)), trace=trace)


def kernel(**inputs):
    in_maps = _prep_in_maps(**{k: np.asarray(v) for k, v in inputs.items()})
    res = _run(in_maps, trace=False)
    return _unpack_out(res.results[0]["out"])


def kernel_traced(**inputs):
    """Like kernel() but with NTFF profiling; returns ((arch, arch_hp), exec_time_ns)."""
    try:
        import ntff_hook
        ntff_hook.install()
    except Exception:
        pass
    in_maps = _prep_in_maps(**{k: np.asarray(v) for k, v in inputs.items()})
    res = _run(in_maps, trace=True)
    return _unpack_out(res.results[0]["out"]), res.exec_time_ns


# revision 20
# speedup vs baseline: 1.2137x; 1.2137x over previous
"""Trainium2 Bass kernel for nn_ArchDecoder: two stacked LSTM cells (H=2048,
H=4096) unrolled DEPTH=12 sequential steps, batch=1, tensor-parallel across 8
NeuronCores.

v3 design:
- Two AllGathers per step: a small A-gather carrying the arch-LSTM state
  [h_a | arch-logit-partials] (the arch chain runs ~2 steps ahead), and a
  B-gather carrying [c_hp | h_hp | hp-logit-partials].
- Deferred softmax normalization: unnormalized exp(logits) feed the next
  step's matvecs (linear); separate PSUM groups are scaled by 1/sum at
  combine time, so the cross-partition sum/reciprocal is off the critical
  path.
- No Sigmoid: sigma(x) = 0.5*tanh(0.5x)+0.5, so exp/tanh/relu share one ACT
  table -> zero ACT_TABLE_LOAD per step.
- PE stream per step: [g2a, g2b, whpc, wsum, ga(t+2) m0..3, lhp, ga m4..7 +
  ga2, woa] — the hp-input matvecs run before whpc so the gate combine only
  gates on whpc's end; the arch block is split around lhp so the payload DMA
  fires as soon as the hp tail is done while the PE keeps crunching arch
  work into the collective's latency shadow.
- Gate-combine partial sums (inp-scale + bias) are emitted before whpc so
  only one DVE add sits after the whpc stream.
- PSUM tiles are never shared across producer constructs with different
  completion times (avoids false waits on the payload path).
- whpc is loaded as 4 m-range pieces and wa as 2, spread over all three
  DMA-capable queues, so early steps overlap the 26MB weight load; a dummy
  collective fires first to absorb the ~50us cold-CC barrier.
- Outputs accumulate in SBUF; one contiguous [128, 48] DMA at the end.

Gate packing order is [i, f, o, g] so the three sigmoids are contiguous.
"""
import sys

for _p in ("/opt/trn_rl_repo", "/root/.axon_site", "/root/.axon_site/_ro/pypackages"):
    if _p not in sys.path:
        sys.path.insert(0, _p)

import numpy as np
import ml_dtypes

import concourse.bass as bass
import concourse.bacc as bacc
import concourse.mybir as mybir
import concourse.tile as tile
from concourse import bass_isa
from concourse.bass_utils import run_bass_kernel_spmd

NC = 8
V = 256
HA = 2048
HHP = 4096
DEPTH = 12
BF = mybir.dt.bfloat16
F32 = mybir.dt.float32
FP8 = mybir.dt.float8e4
AF = mybir.ActivationFunctionType
ALU = mybir.AluOpType

SA = HA // NC          # 256 h_a positions per core
SS = HA // NC          # 256 h_sum positions per core
SHP = SA + SS          # 512 hp-state positions per core
MA = 4 * SA // 128     # 8  M-tiles for arch gates
MHP = 4 * SHP // 128   # 16 M-tiles for hp gates
KA = (V + HA) // 128   # 18 K-chunks for arch gates ([a ; h_a])
KHP_C = HHP // 128     # 32 c_hp K-chunks
KHP_I = (2 * V) // 128 # 4 inp K-chunks
KSUM = HHP // 128      # 32
CA = 4                 # A payload cols: [h_a(2) | lpA(2)]
CB = 10                # B payload cols: [c(4) | h(4) | lpB(2)]
NPC = 4                # whpc split into 4 m-range pieces
MPP = MHP // NPC       # 4 m-tiles per whpc piece

_acol = lambda kc: (kc // 2) * CA + (kc % 2)
_ccol = lambda kc: (kc // 4) * CB + (kc % 4)
_hcol = lambda kc: (kc // 4) * CB + 4 + (kc % 4)


def _build_nc():
    nc = bacc.Bacc(None, target_bir_lowering=False, num_devices=NC)

    wa_e = [nc.declare_dram_parameter(f"wa{p}", [128, 4 * KA * 128], BF,
                                      isOutput=False) for p in range(2)]
    whpc_e = [nc.declare_dram_parameter(f"whpc{p}", [128, MPP * KHP_C * 128], FP8,
                                        isOutput=False) for p in range(NPC)]
    wsum_e = nc.declare_dram_parameter("wsum", [128, 2 * KSUM * 128], BF, isOutput=False)
    whpi_e = nc.declare_dram_parameter("whpi", [128, MHP * KHP_I * 128], BF, isOutput=False)
    woa_e = nc.declare_dram_parameter("woa", [128, 2 * 2 * 128], BF, isOutput=False)
    wohp_e = nc.declare_dram_parameter("wohp", [128, 2 * 4 * 128], BF, isOutput=False)
    ba_e = nc.declare_dram_parameter("ba", [128, MA], F32, isOutput=False)
    bsum_e = nc.declare_dram_parameter("bsum", [128, 2], F32, isOutput=False)
    bhp_e = nc.declare_dram_parameter("bhp", [128, MHP], F32, isOutput=False)
    boa8_e = nc.declare_dram_parameter("boa8", [128, 2], F32, isOutput=False)
    bohp8_e = nc.declare_dram_parameter("bohp8", [128, 2], F32, isOutput=False)
    ident_e = nc.declare_dram_parameter("ident", [128, 128], BF, isOutput=False)
    initA_e = nc.declare_dram_parameter("initA", [128, NC * CA], BF, isOutput=False)
    initB_e = nc.declare_dram_parameter("initB", [128, NC * CB], BF, isOutput=False)
    out_e = nc.declare_dram_parameter("out", [128, 4 * DEPTH], F32, isOutput=True)

    with tile.TileContext(nc, num_cores=NC) as tc:
        with (
            tc.tile_pool(name="wpool", bufs=1) as wpool,
            tc.tile_pool(name="cpool", bufs=1) as cpool,
            tc.tile_pool(name="apool", bufs=4) as apool,
            tc.tile_pool(name="bpool", bufs=4) as bpool,
            tc.tile_pool(name="spool", bufs=4) as spool,
            tc.tile_pool(name="xpool", bufs=3) as xpool,
            tc.tile_pool(name="psA", bufs=1, space="PSUM") as psA,
            tc.tile_pool(name="psGH", bufs=2, space="PSUM") as psGH,
            tc.tile_pool(name="psI", bufs=1, space="PSUM") as psI,
            tc.tile_pool(name="psT", bufs=1, space="PSUM") as psT,
            tc.tile_pool(name="dramA", bufs=3, space="DRAM") as dramA,
            tc.tile_pool(name="dramB", bufs=3, space="DRAM") as dramB,
            tc.tile_pool(name="dramW", bufs=1, space="DRAM") as dramW,
        ):
            # ---- warm collectives first: their tiny input DMA is issued
            # before any weight DMA, so the triggers fire within ~1us and the
            # ~50us cold-CC barrier overlaps the weight load ----
            warm = cpool.tile([1, 16], BF, tag="warm")
            nc.vector.memset(warm[:], 0.0)
            warm_d = dramW.tile([1, 16], BF, tag="warm_d")
            nc.sync.dma_start(warm_d[:], warm[:])
            gw_d = dramW.tile([NC, 1, 16], BF, tag="gw_d")
            nc.gpsimd.collective_compute(
                "AllGather", ALU.bypass,
                replica_groups=[list(range(NC))],
                ins=[warm_d.opt()], outs=[gw_d.opt()],
            )

            # ---- weights: pieces spread over the three DMA queues ----
            initA_sb = cpool.tile([128, NC * CA], BF, tag="initA")
            initB_sb = cpool.tile([128, NC * CB], BF, tag="initB")
            nc.sync.dma_start(initA_sb[:], initA_e[:])
            nc.sync.dma_start(initB_sb[:], initB_e[:])
            wa = [wpool.tile([128, 4 * KA * 128], BF, tag=f"wa{p}", name=f"wa{p}") for p in range(2)]
            whpc = [wpool.tile([128, MPP * KHP_C * 128], FP8, tag=f"whpc{p}",
                                name=f"whpc{p}") for p in range(NPC)]
            wsum = wpool.tile([128, 2 * KSUM * 128], BF, tag="wsum")
            whpi = wpool.tile([128, MHP * KHP_I * 128], BF, tag="whpi")
            woa = wpool.tile([128, 2 * 2 * 128], BF, tag="woa")
            wohp = wpool.tile([128, 2 * 4 * 128], BF, tag="wohp")
            ba = cpool.tile([128, MA], F32, tag="ba")
            bsum = cpool.tile([128, 2], F32, tag="bsum")
            bhp = cpool.tile([128, MHP], F32, tag="bhp")
            boa8 = cpool.tile([128, 2], F32, tag="boa8")
            bohp8 = cpool.tile([128, 2], F32, tag="bohp8")
            idt = cpool.tile([128, 128], BF, tag="idt")
            nc.sync.dma_start(ba[:], ba_e[:])
            nc.sync.dma_start(bsum[:], bsum_e[:])
            nc.sync.dma_start(bhp[:], bhp_e[:])
            nc.sync.dma_start(boa8[:], boa8_e[:])
            nc.sync.dma_start(bohp8[:], bohp8_e[:])
            nc.sync.dma_start(idt[:], ident_e[:])
            nc.sync.dma_start(woa[:], woa_e[:])
            nc.sync.dma_start(wohp[:], wohp_e[:])
            nc.sync.dma_start(wa[0][:], wa_e[0][:])
            nc.sync.dma_start(wa[1][:], wa_e[1][:])
            nc.sync.dma_start(wsum[:], wsum_e[:])
            nc.sync.dma_start(whpi[:], whpi_e[:])
            nc.sync.dma_start(whpc[0][:], whpc_e[0][:])
            nc.sync.dma_start(whpc[1][:], whpc_e[1][:])
            nc.sync.dma_start(whpc[2][:], whpc_e[2][:])
            nc.sync.dma_start(whpc[3][:], whpc_e[3][:])

            a0_bf = cpool.tile([128, 2], BF, tag="a0_bf")
            ahp0_bf = cpool.tile([128, 2], BF, tag="ahp0_bf")
            one_s = cpool.tile([128, 1], F32, tag="one_s")
            nc.vector.memset(a0_bf[:], 1.0 / V)
            nc.vector.memset(ahp0_bf[:], 1.0 / V)
            nc.vector.memset(one_s[:], 1.0)
            c_a = cpool.tile([128, 2], F32, tag="c_a")
            nc.vector.memset(c_a[:], 0.0)
            outA = cpool.tile([128, 2 * DEPTH], F32, tag="outA")
            outHP = cpool.tile([128, 2 * DEPTH], F32, tag="outHP")

            def A_mm1(ea_bf, allA_prev, m_lo, m_hi, ga=None):
                """ga1 MMs for arch m-tiles [m_lo, m_hi); allocates ga on demand.
                ga tile cols: ga1 [0:8], ga2 [8:16], la [16:18]."""
                if ga is None:
                    ga = psA.tile([128, 2 * MA + 2], F32, tag="ga")
                for m in range(m_lo, m_hi):
                    w = wa[m // 4]
                    mm = m % 4
                    for kc in range(16):
                        nc.tensor.matmul(
                            ga[:, m:m + 1],
                            w[:, (mm * KA + 2 + kc) * 128:(mm * KA + 2 + kc + 1) * 128],
                            allA_prev[:, _acol(kc):_acol(kc) + 1],
                            start=(kc == 0), stop=(kc == 15),
                        )
                return ga

            def A_rest(t, ga, ea_bf, Ra, allA_prev):
                """ga2 MMs, arch acts, payA, woa, gather-A(t)."""
                for m in range(MA):
                    w = wa[m // 4]
                    mm = m % 4
                    for j in range(2):
                        nc.tensor.matmul(
                            ga[:, MA + m:MA + m + 1],
                            w[:, (mm * KA + j) * 128:(mm * KA + j + 1) * 128],
                            ea_bf[:, j:j + 1],
                            start=(j == 0), stop=(j == 1),
                        )
                g2s = spool.tile([128, MA], F32, tag="g2s")
                nc.vector.tensor_scalar(g2s[:], ga[:, MA:2 * MA], Ra[:, 0:1], None, ALU.mult)
                nc.vector.tensor_add(g2s[:], g2s[:], ba[:])
                gsb = spool.tile([128, MA], F32, tag="gsb")
                nc.vector.tensor_add(gsb[:], g2s[:], ga[:, 0:MA])
                # gate cols: i=[0:2] f=[2:4] o=[4:6] g=[6:8]
                th = spool.tile([128, MA], F32, tag="th")
                nc.scalar.activation(th[:, 0:6], gsb[:, 0:6], AF.Tanh, scale=0.5)
                nc.scalar.activation(th[:, 6:8], gsb[:, 6:8], AF.Tanh)
                sg = spool.tile([128, 6], F32, tag="sg")
                nc.vector.tensor_scalar(sg[:], th[:, 0:6], 0.5, 0.5, ALU.mult, ALU.add)
                t1a = spool.tile([128, 2], F32, tag="t1a")
                t2a = spool.tile([128, 2], F32, tag="t2a")
                nc.vector.tensor_mul(t1a[:], sg[:, 0:2], th[:, 6:8])
                nc.vector.tensor_mul(t2a[:], sg[:, 2:4], c_a[:])
                nc.vector.tensor_add(c_a[:], t1a[:], t2a[:])
                tc_a = spool.tile([128, 2], F32, tag="tc_a")
                nc.scalar.activation(tc_a[:], c_a[:], AF.Tanh)
                ha_f = apool.tile([128, 2], F32, tag="ha_f")
                nc.vector.tensor_mul(ha_f[:], sg[:, 4:6], tc_a[:])
                payA = apool.tile([128, CA], BF, tag="payA")
                nc.vector.tensor_copy(payA[:, 0:2], ha_f[:])
                for m in range(2):
                    for kc in range(2):
                        nc.tensor.matmul(
                            ga[:, 2 * MA + m:2 * MA + m + 1],
                            woa[:, (m * 2 + kc) * 128:(m * 2 + kc + 1) * 128],
                            payA[:, kc:kc + 1], start=(kc == 0), stop=(kc == 1),
                        )
                nc.vector.tensor_add(payA[:, 2:4], ga[:, 2 * MA:2 * MA + 2], boa8[:])
                tpa = psA.tile([128, 160], BF, tag="tpa")
                tpSA = tpa[0:CA, 32:160]
                nc.tensor.transpose(tpSA, payA[:], idt[:])
                pat = apool.tile([CA, 128], BF, tag="pat")
                nc.vector.tensor_copy(pat[:], tpSA)
                payA_d = dramA.tile([CA, 128], BF, tag="payA_d")
                gA_d = dramA.tile([NC, CA, 128], BF, tag="gA_d")
                nc.sync.dma_start(payA_d[:], pat[:])
                nc.gpsimd.collective_compute(
                    "AllGather", ALU.bypass,
                    replica_groups=[list(range(NC))],
                    ins=[payA_d.opt()], outs=[gA_d.opt()],
                )
                return ha_f, gA_d

            def readback_A(gA_d):
                rbA = xpool.tile([NC * CA, 128], BF, tag="rbA")
                nc.sync.dma_start(rbA[:], gA_d[:].rearrange("r c p -> (r c) p"))
                tpa = psA.tile([128, 160], BF, tag="tpa")
                tpRA = tpa[:, 0:NC * CA]
                nc.tensor.transpose(tpRA, rbA[:], idt[0:NC * CA, 0:NC * CA])
                allA = xpool.tile([128, NC * CA], BF, tag="allA")
                nc.vector.tensor_copy(allA[:], tpRA)
                return allA

            def H2(t, allA_t):
                """exp of arch logits t (unnormalized), 1/sum, outA[:, t]."""
                red = spool.tile([128, 2], F32, tag="redA")
                nc.vector.tensor_reduce(
                    red[:],
                    allA_t[:].rearrange("p (r c) -> p c r", r=NC)[:, 2:4, :],
                    mybir.AxisListType.X, ALU.add,
                )
                e_f = spool.tile([128, 2], F32, tag="eA_f")
                sp = spool.tile([128, 1], F32, tag="sA_p")
                nc.scalar.activation(e_f[:], red[:], AF.Exp, accum_out=sp[:])
                ea_bf = apool.tile([128, 2], BF, tag="ea_bf")
                nc.vector.tensor_copy(ea_bf[:], e_f[:])
                ssum = spool.tile([128, 1], F32, tag="sA_t")
                nc.gpsimd.partition_all_reduce(ssum[:], sp[:], 128, bass_isa.ReduceOp.add)
                Ra = apool.tile([128, 1], F32, tag="Ra")
                nc.vector.reciprocal(Ra[:], ssum[:])
                nc.vector.tensor_scalar(
                    outA[:, 2 * t:2 * t + 2], e_f[:], Ra[:, 0:1], None, ALU.mult)
                return ea_bf, Ra

            def H1(t, allB_t):
                """exp of hp logits t (unnormalized), 1/sum, outHP[:, t]."""
                red = spool.tile([128, 2], F32, tag="redB")
                nc.vector.tensor_reduce(
                    red[:],
                    allB_t[:].rearrange("p (r c) -> p c r", r=NC)[:, 8:10, :],
                    mybir.AxisListType.X, ALU.add,
                )
                e_f = spool.tile([128, 2], F32, tag="eB_f")
                sp = spool.tile([128, 1], F32, tag="sB_p")
                nc.scalar.activation(e_f[:], red[:], AF.Exp, accum_out=sp[:])
                ehp_bf = bpool.tile([128, 2], BF, tag="ehp_bf")
                nc.vector.tensor_copy(ehp_bf[:], e_f[:])
                ssum = spool.tile([128, 1], F32, tag="sB_t")
                nc.gpsimd.partition_all_reduce(ssum[:], sp[:], 128, bass_isa.ReduceOp.add)
                Rhp = bpool.tile([128, 1], F32, tag="Rhp")
                nc.vector.reciprocal(Rhp[:], ssum[:])
                nc.vector.tensor_scalar(
                    outHP[:, 2 * t:2 * t + 2], e_f[:], Rhp[:, 0:1], None, ALU.mult)
                return ehp_bf, Rhp

            def B_mm_head(t, allB_prev, ea_bf, ehp_bf):
                """PE: g2a, g2b, wsum, then whpc m-tiles [0, 8)."""
                gi = psI.tile([128, 2 * MHP], F32, tag="gi")
                for m in range(MHP):
                    for j in range(2):
                        nc.tensor.matmul(
                            gi[:, m:m + 1],
                            whpi[:, (m * KHP_I + j) * 128:(m * KHP_I + j + 1) * 128],
                            ea_bf[:, j:j + 1], start=(j == 0), stop=(j == 1),
                        )
                for m in range(MHP):
                    for j in range(2):
                        nc.tensor.matmul(
                            gi[:, MHP + m:MHP + m + 1],
                            whpi[:, (m * KHP_I + 2 + j) * 128:(m * KHP_I + 2 + j + 1) * 128],
                            ehp_bf[:, j:j + 1], start=(j == 0), stop=(j == 1),
                        )
                ghpA = psGH.tile([128, 8], F32, tag="ghpA")
                ghpB = psGH.tile([128, 12], F32, tag="ghpB")
                for m in range(2):
                    for kc in range(KSUM):
                        nc.tensor.matmul(
                            ghpB[:, 8 + m:9 + m],
                            wsum[:, (m * KSUM + kc) * 128:(m * KSUM + kc + 1) * 128],
                            allB_prev[:, _hcol(kc):_hcol(kc) + 1],
                            start=(kc == 0), stop=(kc == KSUM - 1),
                        )
                if t > 0:
                    B_whpc_half(t, allB_prev, ghpA, 0)
                return ghpA, ghpB, gi

            def B_whpc_half(t, allB_prev, ghp, h):
                for m in range(8 * h, 8 * h + 8):
                    w = whpc[m // MPP]
                    mm = m % MPP
                    for kc in range(KHP_C):
                        nc.tensor.matmul(
                            ghp[:, m - 8 * h:m - 8 * h + 1],
                            w[:, (mm * KHP_C + kc) * 128:(mm * KHP_C + kc + 1) * 128],
                            allB_prev[:, _ccol(kc):_ccol(kc) + 1],
                            start=(kc == 0), stop=(kc == KHP_C - 1),
                        )

            def B_relu(ghpB):
                hs_sb = bpool.tile([128, 2], F32, tag="hs_sb")
                nc.scalar.activation(hs_sb[:, 0:1], ghpB[:, 8:9], AF.Relu,
                                     bias=bsum[:, 0:1])
                nc.scalar.activation(hs_sb[:, 1:2], ghpB[:, 9:10], AF.Relu,
                                     bias=bsum[:, 1:2])
                return hs_sb

            def B_acts_half(t, ghp, gi, ha_f, hs_sb, Ra, Rhp, payB, con, h):
                """hp gate tail for state cols [2h:2h+2] (gates packed
                half-major: m-tiles [8h:8h+8] = [i(2) f(2) o(2) g(2)])."""
                o = 8 * h
                u1 = spool.tile([128, 8], F32, tag=f"u1_{h}", name=f"u1_{h}")
                u2 = spool.tile([128, 8], F32, tag=f"u2_{h}", name=f"u2_{h}")
                nc.vector.tensor_scalar(u1[:], gi[:, o:o + 8], Ra[:, 0:1],
                                        None, ALU.mult)
                nc.vector.tensor_scalar(u2[:], gi[:, MHP + o:MHP + o + 8],
                                        Rhp[:, 0:1], None, ALU.mult)
                gsb = spool.tile([128, 8], F32, tag=f"gsb_{h}", name=f"gsb_{h}")
                nc.vector.tensor_add(gsb[:], u1[:], u2[:])
                nc.vector.tensor_add(gsb[:], gsb[:], bhp[:, o:o + 8])
                if t > 0:
                    nc.vector.tensor_add(gsb[:], gsb[:], ghp[:, 0:8])
                # half-local gate cols: i=[0:2] f=[2:4] o=[4:6] g=[6:8]
                th = spool.tile([128, 8], F32, tag=f"th_{h}", name=f"th_{h}")
                nc.scalar.activation(th[:, 0:6], gsb[:, 0:6], AF.Tanh, scale=0.5)
                nc.scalar.activation(th[:, 6:8], gsb[:, 6:8], AF.Tanh)
                sg = spool.tile([128, 6], F32, tag=f"sg_{h}", name=f"sg_{h}")
                nc.vector.tensor_scalar(sg[:], th[:, 0:6], 0.5, 0.5,
                                        ALU.mult, ALU.add)
                t1 = spool.tile([128, 2], F32, tag=f"t1_{h}", name=f"t1_{h}")
                t2 = spool.tile([128, 2], F32, tag=f"t2_{h}", name=f"t2_{h}")
                nc.vector.tensor_mul(t1[:], sg[:, 0:2], th[:, 6:8])
                nc.vector.tensor_mul(t2[:], sg[:, 2:4],
                                     ha_f[:] if h == 0 else hs_sb[:])
                nc.vector.tensor_add(con[:, 2 * h:2 * h + 2], t1[:], t2[:])
                nc.vector.tensor_copy(payB[:, 2 * h:2 * h + 2],
                                      con[:, 2 * h:2 * h + 2])
                tch = spool.tile([128, 2], F32, tag=f"tch_{h}", name=f"tch_{h}")
                nc.scalar.activation(tch[:], con[:, 2 * h:2 * h + 2], AF.Tanh)
                nc.vector.tensor_mul(payB[:, 4 + 2 * h:6 + 2 * h],
                                     sg[:, 4:6], tch[:])

            def B_fin(t, payB, ghpB):
                """lhp logits, transpose payB to [CB,128], send, gather-B(t)."""
                for m in range(2):
                    for kc in range(4):
                        nc.tensor.matmul(
                            ghpB[:, 10 + m:11 + m],
                            wohp[:, (m * 4 + kc) * 128:(m * 4 + kc + 1) * 128],
                            payB[:, 4 + kc:5 + kc], start=(kc == 0), stop=(kc == 3),
                        )
                nc.vector.tensor_add(payB[:, 8:10], ghpB[:, 10:12], bohp8[:])
                tp = psT.tile([128, 208], BF, tag="tp")
                tpS = tp[0:CB, 80:208]
                nc.tensor.transpose(tpS, payB[:], idt[:])
                pbt = bpool.tile([CB, 128], BF, tag="pbt")
                nc.vector.tensor_copy(pbt[:], tpS)
                payB_d = dramB.tile([CB, 128], BF, tag="payB_d")
                gB_d = dramB.tile([NC, CB, 128], BF, tag="gB_d")
                nc.sync.dma_start(payB_d[:], pbt[:])
                nc.gpsimd.collective_compute(
                    "AllGather", ALU.bypass,
                    replica_groups=[list(range(NC))],
                    ins=[payB_d.opt()], outs=[gB_d.opt()],
                )
                return gB_d

            def readback_B(gB_d):
                rb = xpool.tile([NC * CB, 128], BF, tag="rb")
                nc.sync.dma_start(rb[:], gB_d[:].rearrange("r c p -> (r c) p"))
                tp = psT.tile([128, 208], BF, tag="tp")
                tpR = tp[:, 0:NC * CB]
                nc.tensor.transpose(tpR, rb[:], idt[0:NC * CB, 0:NC * CB])
                allB = xpool.tile([128, NC * CB], BF, tag="allB")
                nc.vector.tensor_copy(allB[:], tpR)
                return allB

            # ---- prologue: arch chain two steps ahead ----
            haf = {}
            eaD = {}
            RaD = {}
            ga0 = A_mm1(a0_bf, initA_sb, 0, MA)
            haf[0], gA_d0 = A_rest(0, ga0, a0_bf, one_s, initA_sb)
            allA_cur = readback_A(gA_d0)
            eaD[0], RaD[0] = H2(0, allA_cur)
            ga1_ = A_mm1(eaD[0], allA_cur, 0, MA)
            haf[1], gA_pending = A_rest(1, ga1_, eaD[0], RaD[0], allA_cur)

            allB_prev = initB_sb
            ehp, Rhp = ahp0_bf, one_s
            for t in range(DEPTH):
                ghpA, ghpB, gi = B_mm_head(t, allB_prev, eaD[t], ehp)
                hs_sb = B_relu(ghpB)
                payB = bpool.tile([128, CB], BF, tag="payB")
                con = bpool.tile([128, 4], F32, tag="con")
                B_acts_half(t, ghpA, gi, haf[t], hs_sb, RaD[t], Rhp, payB, con, 0)
                if gA_pending is not None:
                    allA_cur = readback_A(gA_pending)
                    gA_pending = None
                if t > 0:
                    B_whpc_half(t, allB_prev, ghpB, 1)
                if t + 1 < DEPTH:
                    eaD[t + 1], RaD[t + 1] = H2(t + 1, allA_cur)
                B_acts_half(t, ghpB, gi, haf[t], hs_sb, RaD[t], Rhp, payB, con, 1)
                ga = None
                if t + 2 < DEPTH:
                    ga = A_mm1(eaD[t + 1], allA_cur, 0, 4)
                gB_d = B_fin(t, payB, ghpB)
                if t + 2 < DEPTH:
                    ga = A_mm1(eaD[t + 1], allA_cur, 4, MA, ga=ga)
                    haf[t + 2], gA_pending = A_rest(
                        t + 2, ga, eaD[t + 1], RaD[t + 1], allA_cur)
                allB_new = readback_B(gB_d)
                ehp, Rhp = H1(t, allB_new)
                allB_prev = allB_new

            nc.sync.dma_start(out_e[:, 0:2 * DEPTH], outA[:])
            nc.sync.dma_start(out_e[:, 2 * DEPTH:4 * DEPTH], outHP[:])
    nc.finalize()
    return nc


_NC_CACHE = None


def _get_nc():
    global _NC_CACHE
    if _NC_CACHE is None:
        _NC_CACHE = _build_nc()
    return _NC_CACHE


def _lhsT_pack(w_cat, n_m, n_k):
    """w_cat [n_m*128 rows, n_k*128 cols] -> SBUF image [128, n_m*n_k*128] where
    cols [(m*n_k+kc)*128 + j] on partition p = w_cat[m*128 + j, kc*128 + p]."""
    a = w_cat.reshape(n_m, 128, n_k, 128)           # [m, j, kc, p]
    return np.ascontiguousarray(a.transpose(3, 0, 2, 1).reshape(128, n_m * n_k * 128))


GO = (0, 1, 3, 2)  # pack gate blocks in order i, f, o, g


def _prep_in_maps(x_thought_vec_arch, x_thought_vec_arch_hp,
                  W_ih_a, W_hh_a, b_ih_a, b_hh_a, W_out_a, b_out_a,
                  W_sum, b_sum, W_ih_hp, W_hh_hp, b_ih_hp, b_hh_hp,
                  W_out_hp, b_out_hp):
    f32 = np.float32
    bf16 = ml_dtypes.bfloat16
    php = np.concatenate([
        np.concatenate([np.arange(SA * k, SA * (k + 1)),
                        HA + np.arange(SS * k, SS * (k + 1))])
        for k in range(NC)
    ])
    ba_full = (np.asarray(b_ih_a) + np.asarray(b_hh_a)).astype(f32)
    bhp_full = (np.asarray(b_ih_hp) + np.asarray(b_hh_hp)).astype(f32)
    ha0 = np.asarray(x_thought_vec_arch, f32).reshape(HA)
    hhp0 = np.asarray(x_thought_vec_arch_hp, f32).reshape(HHP)
    W_ih_a = np.asarray(W_ih_a, f32); W_hh_a = np.asarray(W_hh_a, f32)
    W_out_a = np.asarray(W_out_a, f32); W_sum = np.asarray(W_sum, f32)
    W_ih_hp = np.asarray(W_ih_hp, f32); W_hh_hp = np.asarray(W_hh_hp, f32)
    W_out_hp = np.asarray(W_out_hp, f32)
    b_out_a = np.asarray(b_out_a, f32); b_out_hp = np.asarray(b_out_hp, f32)
    b_sum = np.asarray(b_sum, f32)

    initA = np.zeros((128, NC * CA), f32)
    initB = np.zeros((128, NC * CB), f32)
    hhp0_p = hhp0[php]
    for r in range(NC):
        for q in range(2):
            initA[:, r * CA + q] = ha0[r * SA + q * 128: r * SA + (q + 1) * 128]
        for q in range(4):
            initB[:, r * CB + 4 + q] = hhp0_p[r * SHP + q * 128: r * SHP + (q + 1) * 128]

    in_maps = []
    for k in range(NC):
        ja = np.arange(SA * k, SA * (k + 1))
        rows_a = np.concatenate([g * HA + ja for g in GO])
        wa_cat = np.concatenate([W_ih_a[rows_a], W_hh_a[rows_a]], axis=1)
        jhp = php[SHP * k: SHP * (k + 1)]
        # half-major gate packing: [i f o g] for state cols 0:2, then for 2:4
        rows_hp = np.concatenate([g * HHP + jhp[256 * h: 256 * (h + 1)]
                                  for h in range(2) for g in GO])
        whpc_cat = W_hh_hp[rows_hp][:, php]
        whpi_cat = W_ih_hp[rows_hp]
        js = np.arange(SS * k, SS * (k + 1))
        wsum_p = W_sum[js][:, php]
        woa_p = W_out_a[:, ja]
        wohp_p = W_out_hp[:, jhp]
        im = {
            "wsum": _lhsT_pack(wsum_p, 2, KSUM).astype(bf16),
            "whpi": _lhsT_pack(whpi_cat, MHP, KHP_I).astype(bf16),
            "woa": _lhsT_pack(woa_p, 2, 2).astype(bf16),
            "wohp": _lhsT_pack(wohp_p, 2, 4).astype(bf16),
            "ba": np.ascontiguousarray(ba_full[rows_a].reshape(MA, 128).T),
            "bsum": np.ascontiguousarray(b_sum[js].reshape(2, 128).T),
            "bhp": np.ascontiguousarray(bhp_full[rows_hp].reshape(MHP, 128).T),
            "boa8": np.ascontiguousarray((b_out_a / NC).reshape(2, 128).T),
            "bohp8": np.ascontiguousarray((b_out_hp / NC).reshape(2, 128).T),
            "ident": np.eye(128, dtype=bf16),
            "initA": initA.astype(bf16),
            "initB": initB.astype(bf16),
        }
        for p in range(2):
            im[f"wa{p}"] = _lhsT_pack(
                wa_cat[p * 512:(p + 1) * 512], 4, KA).astype(bf16)
        fp8 = ml_dtypes.float8_e4m3
        for p in range(NPC):
            im[f"whpc{p}"] = np.clip(_lhsT_pack(
                whpc_cat[p * MPP * 128:(p + 1) * MPP * 128], MPP, KHP_C),
                -240, 240).astype(fp8)
        in_maps.append(im)
    return in_maps


def _unpack_out(out):
    """out [128, 4*DEPTH] f32 -> (arch [1, DEPTH, V], arch_hp [1, DEPTH, V])."""
    out = np.asarray(out, np.float32)
    arch = out[:, :2 * DEPTH].reshape(128, DEPTH, 2).transpose(1, 2, 0).reshape(DEPTH, V)
    ahp = out[:, 2 * DEPTH:].reshape(128, DEPTH, 2).transpose(1, 2, 0).reshape(DEPTH, V)
    return arch[None], ahp[None]


def _run(in_maps, trace=False):
    nc = _get_nc()
    return run_bass_kernel_spmd(nc, in_maps, core_ids=list(range(NC)), trace=trace)


def kernel(**inputs):
    in_maps = _prep_in_maps(**{k: np.asarray(v) for k, v in inputs.items()})
    res = _run(in_maps, trace=False)
    return _unpack_out(res.results[0]["out"])


def kernel_traced(**inputs):
    """Like kernel() but with NTFF profiling; returns ((arch, arch_hp), exec_time_ns)."""
    try:
        import ntff_hook
        ntff_hook.install()
    except Exception:
        pass
    in_maps = _prep_in_maps(**{k: np.asarray(v) for k, v in inputs.items()})
    res = _run(in_maps, trace=True)
    return _unpack_out(res.results[0]["out"]), res.exec_time_ns
